# revision 5
# baseline (speedup 1.0000x reference)
"""MoE gate (top-6 routing) Trainium2 Bass kernel.

Problem: hidden_states [4, 4096, 2048] f32, gate weight [64, 2048] f32.
  logits = x @ W.T            -> [16384, 64]
  topk_weight, topk_idx = top_k(logits, 6)
  topk_weight = softmax(topk_weight)   (the reference's extra
  normalization divides by 1.0 + 1e-20 and is a no-op in fp32)
Returns (topk_idx int32 [16384, 6], topk_weight f32 [16384, 6]).

Sharding: data-parallel over tokens. Each of the 8 cores gets 2048
tokens; the gate weight is replicated. The token shard is fed to the
device pre-transposed ([H, T] layout) so the contraction dim lands on
SBUF partitions with fully contiguous DMA loads.

Per-core kernel (all fp32 — bf16/tf32 would flip near-tied expert
rankings vs the fp32 reference):
  - two 1024-token super-panels, streamed panel-major so panel 0's
    top-k overlaps panel 1's DMA
  - matmuls in [E, T'] orientation (w-tile stationary, x moving at
    N=512), 2-way column-tiled: a panel's two 512-token blocks
    accumulate concurrently into partition halves [0:64] / [64:128]
    of one PSUM bank (distinct col-groups -> concurrent streams)
  - PE-transpose of the small logits to [token, expert] tiles
  - DVE max8/max_index (straight from PSUM) -> top-8 + indices
  - ACT exp(v - max) with accumulated sum, DVE reciprocal + scale
  - results staged in SBUF, per-panel DMAs out; host de-interleaves
"""

import numpy as np

import concourse.bass as bass
import concourse.mybir as mybir
import concourse.tile as tile
from concourse import bacc
from concourse.bass_utils import run_bass_kernel_spmd

f32 = mybir.dt.float32
u32 = mybir.dt.uint32
i32 = mybir.dt.int32

N_CORES = 8
B, S, H = 4, 4096, 2048
E = 64
TOP_K = 6
T_FULL = B * S              # 16384 tokens
T_CORE = T_FULL // N_CORES  # 2048 tokens per core
KT = H // 128               # 16 contraction tiles
NTT = T_CORE // 128         # 16 token tiles per core
TB = 512                    # tokens per matmul block (PSUM bank = 512 fp32)
PANEL = 2 * TB              # 1024 tokens per super-panel (one packed psum)
NP = T_CORE // PANEL        # 2 super-panels per core

_CACHE = {}


def _build():
    nc = bacc.Bacc("TRN2", target_bir_lowering=False, debug=False)
    xT = nc.dram_tensor("xT", [H, T_CORE], f32, kind="ExternalInput").ap()
    wT = nc.dram_tensor("wT", [H, E], f32, kind="ExternalInput").ap()
    ident = nc.dram_tensor("ident", [E, E], f32, kind="ExternalInput").ap()
    out_w = nc.dram_tensor("out_w", [128, NTT * TOP_K], f32, kind="ExternalOutput").ap()
    out_i = nc.dram_tensor("out_i", [128, NTT * 8], i32, kind="ExternalOutput").ap()

    with tile.TileContext(nc) as tc:
        with (
            tc.tile_pool(name="persist", bufs=1) as persist,
            tc.tile_pool(name="work", bufs=4) as work,
            tc.tile_pool(name="psum", bufs=2, space="PSUM") as psp,
            tc.tile_pool(name="psumT", bufs=4, space="PSUM") as pspT,
            tc.tile_pool(name="psumW", bufs=1, space="PSUM") as pspW,
        ):
            # First x tile queued before anything else so the PE can start
            # as early as possible; w next; ident deferred (only needed for
            # the transposes ~30us in).
            all_x = {}

            def load_x(q, a):
                xt = persist.tile([128, PANEL], f32, tag=f"x{q}_{a}")
                nc.sync.dma_start(
                    out=xt,
                    in_=xT[a * 128 : (a + 1) * 128, q * PANEL : (q + 1) * PANEL],
                )
                all_x[(q, a)] = xt

            load_x(0, 0)
            w_all = persist.tile([128, KT * E], f32, tag="w_all")
            nc.sync.dma_start(
                out=w_all.rearrange("p (a e) -> p a e", a=KT),
                in_=wT.rearrange("(a p) e -> p a e", p=128),
            )
            for a in range(1, KT):
                load_x(0, a)
            id_t = persist.tile([E, E], f32, tag="ident")
            nc.sync.dma_start(out=id_t, in_=ident)
            for a in range(KT):
                load_x(1, a)

            # Warmup matmuls: absorb the w DMA wait on the PE (the fused fp32
            # matmul carries at most one semaphore wait) and spin the PE so
            # the HAM clock-gate warms before the real matmuls arrive.
            ps_warm = pspW.tile([64, 64], f32, tag="ps_warm")
            for _ in range(8):
                nc.tensor.matmul(
                    ps_warm, w_all[:, 0:64], w_all[:, 0:64], start=True, stop=True
                )
            # absorb the ident DMA wait + warm the transpose path
            nc.tensor.transpose(ps_warm, id_t, id_t)

            stage_w = persist.tile([128, NTT * TOP_K], f32, tag="stage_w")
            stage_i = persist.tile([128, NTT * 8], u32, tag="stage_i")

            for q in range(NP):
                x_tiles = [all_x[(q, a)] for a in range(KT)]

                # ---- packed accumulation: block half=0 -> partitions 0:64,
                #      half=1 -> partitions 64:128 (concurrent col-groups) ----
                ps = psp.tile([128, TB], f32, tag="ps")
                for a in range(KT):
                    w_tile = w_all[:, a * E : (a + 1) * E]
                    for half in range(2):
                        nc.tensor.matmul(
                            ps[half * 64 : (half + 1) * 64, :],
                            w_tile,
                            x_tiles[a][:, half * TB : (half + 1) * TB],
                            start=(a == 0),
                            stop=(a == KT - 1),
                        )

                # ---- per-block epilogue: copy logits.T to SBUF, transpose,
                #      top-k + softmax per 128-token tile ----
                for half in range(2):
                    ltE = work.tile([64, TB], f32, tag="ltE")
                    nc.scalar.copy(
                        out=ltE, in_=ps[half * 64 : (half + 1) * 64, :]
                    )
                    for tt in range(TB // 128):
                        t = (2 * q + half) * (TB // 128) + tt
                        ps_t = pspT.tile([128, E], f32, tag="ps_t")
                        nc.tensor.transpose(
                            ps_t, ltE[:, tt * 128 : (tt + 1) * 128], id_t
                        )

                        m8 = work.tile([128, 8], f32, tag="m8")
                        nc.vector.max(out=m8, in_=ps_t)
                        nc.vector.max_index(
                            stage_i[:, t * 8 : (t + 1) * 8], m8, ps_t
                        )

                        negm = work.tile([128, 1], f32, tag="negm")
                        nc.scalar.mul(negm, m8[:, 0:1], -1.0)
                        expw = work.tile([128, TOP_K], f32, tag="expw")
                        ssum = work.tile([128, 1], f32, tag="ssum")
                        nc.scalar.activation(
                            out=expw,
                            in_=m8[:, 0:TOP_K],
                            func=mybir.ActivationFunctionType.Exp,
                            bias=negm[:, 0:1],
                            scale=1.0,
                            accum_out=ssum[:, 0:1],
                        )
                        rsum = work.tile([128, 1], f32, tag="rsum")
                        nc.vector.reciprocal(rsum, ssum)
                        nc.vector.tensor_scalar_mul(
                            stage_w[:, t * TOP_K : (t + 1) * TOP_K],
                            expw,
                            rsum[:, 0:1],
                        )

                    # ---- per-half-panel output DMAs ----
                    nt_h = TB // 128  # 4 token tiles per half
                    c0 = (2 * q + half) * nt_h
                    nc.gpsimd.dma_start(
                        out=out_w[:, c0 * TOP_K : (c0 + nt_h) * TOP_K],
                        in_=stage_w[:, c0 * TOP_K : (c0 + nt_h) * TOP_K],
                    )
                    nc.gpsimd.dma_start(
                        out=out_i[:, c0 * 8 : (c0 + nt_h) * 8],
                        in_=stage_i[:, c0 * 8 : (c0 + nt_h) * 8].bitcast(i32),
                    )


    nc.compile()
    return nc


def _get_nc():
    if "nc" not in _CACHE:
        _CACHE["nc"] = _build()
    return _CACHE["nc"]


def kernel(hidden_states: np.ndarray, weight: np.ndarray, **_run_kwargs):
    x = np.ascontiguousarray(hidden_states, dtype=np.float32).reshape(T_FULL, H)
    w = np.ascontiguousarray(weight, dtype=np.float32)

    wT = np.ascontiguousarray(w.T)  # [H, E]
    ident = np.eye(E, dtype=np.float32)
    in_maps = []
    for c in range(N_CORES):
        shard = x[c * T_CORE : (c + 1) * T_CORE, :]  # [T_CORE, H]
        xT = np.ascontiguousarray(shard.T)  # [H, T_CORE]
        in_maps.append({"xT": xT, "wT": wT, "ident": ident})

    nc = _get_nc()
    res = run_bass_kernel_spmd(
        nc, in_maps, core_ids=list(range(N_CORES)), **_run_kwargs
    )

    idx_parts = []
    w_parts = []
    for c in range(N_CORES):
        r = res.results[c]
        # stage_w [128, NTT*6]: row p, col t*6+k -> token t*128+p, slot k
        # stage_i [128, NTT*8]: row p, col t*8+k -> token t*128+p, slot k (k<6)
        si = r["out_i"].reshape(128, NTT, 8).transpose(1, 0, 2)[:, :, :TOP_K]
        sw = r["out_w"].reshape(128, NTT, TOP_K).transpose(1, 0, 2)
        idx_parts.append(si.reshape(T_CORE, TOP_K).astype(np.int32, copy=False))
        w_parts.append(sw.reshape(T_CORE, TOP_K))

    topk_idx = np.concatenate(idx_parts, axis=0)
    topk_weight = np.concatenate(w_parts, axis=0)
    if "trace" in _run_kwargs:
        return (topk_idx, topk_weight), res
    return topk_idx, topk_weight


# revision 6
# speedup vs baseline: 1.0350x; 1.0350x over previous
"""MoE gate (top-6 routing) Trainium2 Bass kernel.

Problem: hidden_states [4, 4096, 2048] f32, gate weight [64, 2048] f32.
  logits = x @ W.T            -> [16384, 64]
  topk_weight, topk_idx = top_k(logits, 6)
  topk_weight = softmax(topk_weight)   (the reference's extra
  normalization divides by 1.0 + 1e-20 and is a no-op in fp32)
Returns (topk_idx int32 [16384, 6], topk_weight f32 [16384, 6]).

Sharding: data-parallel over tokens. Each of the 8 cores gets 2048
tokens; the gate weight is replicated.

Precision scheme (fp32-accurate at half the HBM traffic): each fp32
value is split on the host into two fp16 halves,
    xh = fp16(x),  xl = fp16((x - xh) * 2^11)
so x = xh + 2^-11*xl to ~2^-23 relative precision. Then
    logits = xh@wh.T + 2^-11 * (xh@wl.T + xl@wh.T)      (+O(2^-22) term dropped)
which matches the fp32 reference to below fp32 accumulation noise
(verified: bit-level top-6 agreement with the jax fp32 reference on the
actual test inputs). fp16 matmuls stream at 1 cycle/row (vs 4 for
fp32) and the input stream is 8 MB/core instead of 16.

Per-core kernel:
  - x halves fed pre-transposed ([H, T] layout, fp16) so the
    contraction dim lands on SBUF partitions with contiguous DMAs;
    w halves fed pre-packed as [128, 16*64] fp16
  - two 1024-token super-panels streamed panel-major (panel 0's top-k
    overlaps panel 1's DMA); x loaded in 1 MiB chunks of 4 h-tiles
  - matmuls in [E, T'] orientation (w stationary, x moving at N=512),
    2-way column-tiled: a panel's two 512-token blocks accumulate
    concurrently in partition halves [0:64]/[64:128] of PSUM banks
  - combine P1 + 2^-11*P2 (ACT scaled copy + DVE add)
  - PE-transpose of the logits to [token, expert] tiles
  - DVE max8/max_index (from PSUM) -> top-8 values + indices
  - ACT exp(v - max) with accumulated sum, DVE reciprocal + scale
  - results staged in SBUF, per-half-panel DMAs out; host de-interleaves
"""

import numpy as np

import concourse.bass as bass
import concourse.mybir as mybir
import concourse.tile as tile
from concourse import bacc
from concourse.bass_utils import run_bass_kernel_spmd

f32 = mybir.dt.float32
f16 = mybir.dt.float16
u32 = mybir.dt.uint32
i32 = mybir.dt.int32

N_CORES = 8
B, S, H = 4, 4096, 2048
E = 64
TOP_K = 6
T_FULL = B * S              # 16384 tokens
T_CORE = T_FULL // N_CORES  # 2048 tokens per core
KT = H // 128               # 16 contraction tiles
NTT = T_CORE // 128         # 16 token tiles per core
TB = 512                    # tokens per matmul block (PSUM bank = 512 fp32)
PANEL = 2 * TB              # 1024 tokens per super-panel (one packed psum pair)
NP = T_CORE // PANEL        # 2 super-panels per core
CH = 4                      # h-tiles per DMA chunk (1 MiB per chunk per half)
NCH = KT // CH              # 4 chunks per panel
LSCALE = float(2.0 ** -11)

_CACHE = {}


def _build():
    nc = bacc.Bacc("TRN2", target_bir_lowering=False, debug=False)
    xh = nc.dram_tensor("xh", [H, T_CORE], f16, kind="ExternalInput").ap()
    xl = nc.dram_tensor("xl", [H, T_CORE], f16, kind="ExternalInput").ap()
    wh = nc.dram_tensor("wh", [128, KT * E], f16, kind="ExternalInput").ap()
    wl = nc.dram_tensor("wl", [128, KT * E], f16, kind="ExternalInput").ap()
    ident = nc.dram_tensor("ident", [E, E], f32, kind="ExternalInput").ap()
    out_w = nc.dram_tensor("out_w", [128, NTT * TOP_K], f32, kind="ExternalOutput").ap()
    out_i = nc.dram_tensor("out_i", [128, NTT * 8], i32, kind="ExternalOutput").ap()

    with tile.TileContext(nc) as tc:
        with (
            tc.tile_pool(name="persist", bufs=1) as persist,
            tc.tile_pool(name="work", bufs=4) as work,
            tc.tile_pool(name="psum", bufs=2, space="PSUM") as psp,
            tc.tile_pool(name="psumT", bufs=3, space="PSUM") as pspT,
            tc.tile_pool(name="psumW", bufs=1, space="PSUM") as pspW,
        ):
            # ---- weights first (warmups depend on them), then x chunks ----
            wh_all = persist.tile([128, KT * E], f16, tag="wh_all")
            nc.sync.dma_start(out=wh_all, in_=wh)
            wl_all = persist.tile([128, KT * E], f16, tag="wl_all")
            nc.sync.dma_start(out=wl_all, in_=wl)

            xh_ch = {}
            xl_ch = {}

            def load_chunk(q, c):
                th = persist.tile([128, CH * PANEL], f16, tag=f"xh{q}_{c}")
                nc.sync.dma_start(
                    out=th.rearrange("p (j t) -> p j t", j=CH),
                    in_=xh[
                        c * CH * 128 : (c + 1) * CH * 128,
                        q * PANEL : (q + 1) * PANEL,
                    ].rearrange("(j p) t -> p j t", p=128),
                )
                xh_ch[(q, c)] = th
                tl = persist.tile([128, CH * PANEL], f16, tag=f"xl{q}_{c}")
                nc.sync.dma_start(
                    out=tl.rearrange("p (j t) -> p j t", j=CH),
                    in_=xl[
                        c * CH * 128 : (c + 1) * CH * 128,
                        q * PANEL : (q + 1) * PANEL,
                    ].rearrange("(j p) t -> p j t", p=128),
                )
                xl_ch[(q, c)] = tl

            for c in range(NCH):
                load_chunk(0, c)
            id_t = persist.tile([E, E], f32, tag="ident")
            nc.sync.dma_start(out=id_t, in_=ident)
            for c in range(NCH):
                load_chunk(1, c)

            # Warmup matmuls: absorb the wh/wl DMA waits on the PE (a fused
            # matmul carries at most one semaphore wait) and spin the PE so
            # the HAM clock-gate warms before the real matmuls arrive.
            ps_warm = pspW.tile([64, 64], f32, tag="ps_warm")
            for _ in range(6):
                nc.tensor.matmul(
                    ps_warm, wh_all[:, 0:64], wh_all[:, 0:64], start=True, stop=True
                )
            nc.tensor.matmul(
                ps_warm, wl_all[:, 0:64], wl_all[:, 0:64], start=True, stop=True
            )
            # absorb the ident DMA wait + warm the transpose path
            nc.tensor.transpose(ps_warm, id_t, id_t)

            stage_w = persist.tile([128, NTT * TOP_K], f32, tag="stage_w")
            stage_i = persist.tile([128, NTT * 8], u32, tag="stage_i")

            for q in range(NP):
                # ---- packed accumulation; half -> partition range / col-group
                ps1 = psp.tile([128, TB], f32, tag="ps1")  # xh@wh
                ps2 = psp.tile([128, TB], f32, tag="ps2")  # xh@wl + xl@wh
                for a in range(KT):
                    c, j = divmod(a, CH)
                    wh_t = wh_all[:, a * E : (a + 1) * E]
                    wl_t = wl_all[:, a * E : (a + 1) * E]
                    for half in range(2):
                        sl = slice(j * PANEL + half * TB, j * PANEL + (half + 1) * TB)
                        pr = slice(half * 64, (half + 1) * 64)
                        nc.tensor.matmul(
                            ps1[pr, :], wh_t, xh_ch[(q, c)][:, sl],
                            start=(a == 0), stop=(a == KT - 1),
                        )
                        nc.tensor.matmul(
                            ps2[pr, :], wl_t, xh_ch[(q, c)][:, sl],
                            start=(a == 0), stop=False,
                        )
                        nc.tensor.matmul(
                            ps2[pr, :], wh_t, xl_ch[(q, c)][:, sl],
                            start=False, stop=(a == KT - 1),
                        )

                # ---- per-block epilogue ----
                for half in range(2):
                    pr = slice(half * 64, (half + 1) * 64)
                    t2 = work.tile([64, TB], f32, tag="t2")
                    nc.scalar.activation(
                        out=t2,
                        in_=ps2[pr, :],
                        func=mybir.ActivationFunctionType.Copy,
                        scale=LSCALE,
                    )
                    ltE = work.tile([64, TB], f32, tag="ltE")
                    nc.vector.tensor_add(ltE, t2, ps1[pr, :])

                    for tt in range(TB // 128):
                        t = (2 * q + half) * (TB // 128) + tt
                        ps_t = pspT.tile([128, E], f32, tag="ps_t")
                        nc.tensor.transpose(
                            ps_t, ltE[:, tt * 128 : (tt + 1) * 128], id_t
                        )

                        m8 = work.tile([128, 8], f32, tag="m8")
                        nc.vector.max(out=m8, in_=ps_t)
                        nc.vector.max_index(
                            stage_i[:, t * 8 : (t + 1) * 8], m8, ps_t
                        )

                        negm = work.tile([128, 1], f32, tag="negm")
                        nc.scalar.mul(negm, m8[:, 0:1], -1.0)
                        expw = work.tile([128, TOP_K], f32, tag="expw")
                        ssum = work.tile([128, 1], f32, tag="ssum")
                        nc.scalar.activation(
                            out=expw,
                            in_=m8[:, 0:TOP_K],
                            func=mybir.ActivationFunctionType.Exp,
                            bias=negm[:, 0:1],
                            scale=1.0,
                            accum_out=ssum[:, 0:1],
                        )
                        rsum = work.tile([128, 1], f32, tag="rsum")
                        nc.vector.reciprocal(rsum, ssum)
                        nc.vector.tensor_scalar_mul(
                            stage_w[:, t * TOP_K : (t + 1) * TOP_K],
                            expw,
                            rsum[:, 0:1],
                        )

                    # ---- per-half-panel output DMAs ----
                    nt_h = TB // 128  # 4 token tiles per half
                    c0 = (2 * q + half) * nt_h
                    nc.gpsimd.dma_start(
                        out=out_w[:, c0 * TOP_K : (c0 + nt_h) * TOP_K],
                        in_=stage_w[:, c0 * TOP_K : (c0 + nt_h) * TOP_K],
                    )
                    nc.gpsimd.dma_start(
                        out=out_i[:, c0 * 8 : (c0 + nt_h) * 8],
                        in_=stage_i[:, c0 * 8 : (c0 + nt_h) * 8].bitcast(i32),
                    )

    nc.compile()
    return nc


def _get_nc():
    if "nc" not in _CACHE:
        _CACHE["nc"] = _build()
    return _CACHE["nc"]


def _split_fp16(arr32):
    """arr32 (fp32) -> (hi fp16, lo fp16) with arr32 ~= hi + 2^-11 * lo."""
    hi = arr32.astype(np.float16)
    lo = ((arr32 - hi.astype(np.float32)) * 2048.0).astype(np.float16)
    return hi, lo


def kernel(hidden_states: np.ndarray, weight: np.ndarray, **_run_kwargs):
    x = np.ascontiguousarray(hidden_states, dtype=np.float32).reshape(T_FULL, H)
    w = np.ascontiguousarray(weight, dtype=np.float32)

    w_hi, w_lo = _split_fp16(w)  # [E, H] fp16
    # device layout [128, KT*E]: row p, col a*E+e  <-  W[e, a*128+p]
    def pack_w(wx):
        return np.ascontiguousarray(
            wx.T.reshape(KT, 128, E).transpose(1, 0, 2).reshape(128, KT * E)
        )

    whp = pack_w(w_hi)
    wlp = pack_w(w_lo)
    ident = np.eye(E, dtype=np.float32)

    in_maps = []
    for c in range(N_CORES):
        shard = x[c * T_CORE : (c + 1) * T_CORE, :]  # [T_CORE, H]
        xT = np.ascontiguousarray(shard.T)  # [H, T_CORE] fp32
        xhs, xls = _split_fp16(xT)
        in_maps.append(
            {"xh": xhs, "xl": xls, "wh": whp, "wl": wlp, "ident": ident}
        )

    nc = _get_nc()
    res = run_bass_kernel_spmd(
        nc, in_maps, core_ids=list(range(N_CORES)), **_run_kwargs
    )

    idx_parts = []
    w_parts = []
    for c in range(N_CORES):
        r = res.results[c]
        si = r["out_i"].reshape(128, NTT, 8).transpose(1, 0, 2)[:, :, :TOP_K]
        sw = r["out_w"].reshape(128, NTT, TOP_K).transpose(1, 0, 2)
        idx_parts.append(si.reshape(T_CORE, TOP_K).astype(np.int32, copy=False))
        w_parts.append(sw.reshape(T_CORE, TOP_K))

    topk_idx = np.concatenate(idx_parts, axis=0)
    topk_weight = np.concatenate(w_parts, axis=0)
    if "trace" in _run_kwargs:
        return (topk_idx, topk_weight), res
    return topk_idx, topk_weight


# revision 7
# speedup vs baseline: 1.0785x; 1.0420x over previous
"""MoE gate (top-6 routing) Trainium2 Bass kernel.

Problem: hidden_states [4, 4096, 2048] f32, gate weight [64, 2048] f32.
  logits = x @ W.T            -> [16384, 64]
  topk_weight, topk_idx = top_k(logits, 6)
  topk_weight = softmax(topk_weight)   (the reference's extra
  normalization divides by 1.0 + 1e-20 and is a no-op in fp32)
Returns (topk_idx int32 [16384, 6], topk_weight f32 [16384, 6]).

Sharding: data-parallel over tokens. Each of the 8 cores gets 2048
tokens; the gate weight is replicated.

Precision scheme (fp32-accurate at half the HBM traffic): each fp32
value is split on the host into two fp16 halves,
    xh = fp16(x),  xl = fp16((x - xh) * 2^11)
so x = xh + 2^-11*xl to ~2^-23 relative precision. Then
    logits = xh@wh.T + 2^-11 * (xh@wl.T + xl@wh.T)      (+O(2^-22) term dropped)
which matches the fp32 reference to below fp32 accumulation noise
(verified: bit-level top-6 agreement with the jax fp32 reference on the
actual test inputs). fp16 matmuls stream at 1 cycle/row (vs 4 for
fp32) and the input stream is 8 MB/core instead of 16.

Per-core kernel:
  - x halves fed pre-transposed ([H, T] layout, fp16) so the
    contraction dim lands on SBUF partitions with contiguous DMAs;
    w halves fed pre-packed as [128, 16*64] fp16
  - two 1024-token super-panels streamed panel-major (panel 0's top-k
    overlaps panel 1's DMA); x loaded in 1 MiB chunks of 4 h-tiles
  - matmuls in [E, T'] orientation (w stationary, x moving at N=512),
    2-way column-tiled: a panel's two 512-token blocks accumulate
    concurrently in partition halves [0:64]/[64:128] of PSUM banks
  - combine P1 + 2^-11*P2 (ACT scaled copy + DVE add)
  - PE-transpose of the logits to [token, expert] tiles
  - DVE max8/max_index (from PSUM) -> top-8 values + indices
  - ACT exp(v - max) with accumulated sum, DVE reciprocal + scale
  - results staged in SBUF, per-half-panel DMAs out; host de-interleaves
"""

import numpy as np

import concourse.bass as bass
import concourse.mybir as mybir
import concourse.tile as tile
from concourse import bacc
from concourse.bass_utils import run_bass_kernel_spmd

f32 = mybir.dt.float32
f16 = mybir.dt.float16
u32 = mybir.dt.uint32
i32 = mybir.dt.int32

N_CORES = 8
B, S, H = 4, 4096, 2048
E = 64
TOP_K = 6
T_FULL = B * S              # 16384 tokens
T_CORE = T_FULL // N_CORES  # 2048 tokens per core
KT = H // 128               # 16 contraction tiles
NTT = T_CORE // 128         # 16 token tiles per core
TB = 512                    # tokens per matmul block (PSUM bank = 512 fp32)
PANEL = 2 * TB              # 1024 tokens per super-panel (one packed psum pair)
NP = T_CORE // PANEL        # 2 super-panels per core
CH = 4                      # h-tiles per DMA chunk (1 MiB per chunk per half)
NCH = KT // CH              # 4 chunks per panel
LSCALE = float(2.0 ** -11)

_CACHE = {}


def _build():
    nc = bacc.Bacc("TRN2", target_bir_lowering=False, debug=False)
    # x halves host-packed per DMA chunk: [NP*NCH, 128, CH*PANEL], where
    # chunk k=q*NCH+c holds h-tiles c*CH..c*CH+CH-1 of panel q.
    xh = nc.dram_tensor("xh", [NP * NCH, 128, CH * PANEL], f16, kind="ExternalInput").ap()
    xl = nc.dram_tensor("xl", [NP * NCH, 128, CH * PANEL], f16, kind="ExternalInput").ap()
    wh = nc.dram_tensor("wh", [128, KT * E], f16, kind="ExternalInput").ap()
    wl = nc.dram_tensor("wl", [128, KT * E], f16, kind="ExternalInput").ap()
    ident = nc.dram_tensor("ident", [E, E], f32, kind="ExternalInput").ap()
    out_w = nc.dram_tensor("out_w", [128, NTT * TOP_K], f32, kind="ExternalOutput").ap()
    out_i = nc.dram_tensor("out_i", [128, NTT * 8], i32, kind="ExternalOutput").ap()

    with tile.TileContext(nc) as tc:
        with (
            tc.tile_pool(name="persist", bufs=1) as persist,
            tc.tile_pool(name="work", bufs=4) as work,
            tc.tile_pool(name="psum", bufs=2, space="PSUM") as psp,
            tc.tile_pool(name="psumT", bufs=3, space="PSUM") as pspT,
            tc.tile_pool(name="psumW", bufs=1, space="PSUM") as pspW,
        ):
            # ---- weights first (warmups depend on them), then x chunks ----
            wh_all = persist.tile([128, KT * E], f16, tag="wh_all")
            nc.sync.dma_start(out=wh_all, in_=wh)
            wl_all = persist.tile([128, KT * E], f16, tag="wl_all")
            nc.sync.dma_start(out=wl_all, in_=wl)

            xh_ch = {}
            xl_ch = {}

            def load_chunk(q, c):
                k = q * NCH + c
                th = persist.tile([128, CH * PANEL], f16, tag=f"xh{q}_{c}")
                nc.sync.dma_start(out=th, in_=xh[k])
                xh_ch[(q, c)] = th
                tl = persist.tile([128, CH * PANEL], f16, tag=f"xl{q}_{c}")
                nc.sync.dma_start(out=tl, in_=xl[k])
                xl_ch[(q, c)] = tl

            for c in range(NCH):
                load_chunk(0, c)
            id_t = persist.tile([E, E], f32, tag="ident")
            nc.sync.dma_start(out=id_t, in_=ident)
            for c in range(NCH):
                load_chunk(1, c)

            # Warmup matmuls: absorb the wh/wl DMA waits on the PE (a fused
            # matmul carries at most one semaphore wait) and spin the PE so
            # the HAM clock-gate warms before the real matmuls arrive.
            ps_warm = pspW.tile([64, 64], f32, tag="ps_warm")
            for _ in range(6):
                nc.tensor.matmul(
                    ps_warm, wh_all[:, 0:64], wh_all[:, 0:64], start=True, stop=True
                )
            nc.tensor.matmul(
                ps_warm, wl_all[:, 0:64], wl_all[:, 0:64], start=True, stop=True
            )
            # absorb the ident DMA wait + warm the transpose path
            nc.tensor.transpose(ps_warm, id_t, id_t)

            stage_w = persist.tile([128, NTT * TOP_K], f32, tag="stage_w")
            stage_i = persist.tile([128, NTT * 8], u32, tag="stage_i")

            for q in range(NP):
                # ---- packed accumulation; half -> partition range / col-group
                ps1 = psp.tile([128, TB], f32, tag="ps1")  # xh@wh
                ps2 = psp.tile([128, TB], f32, tag="ps2")  # xh@wl + xl@wh
                for a in range(KT):
                    c, j = divmod(a, CH)
                    wh_t = wh_all[:, a * E : (a + 1) * E]
                    wl_t = wl_all[:, a * E : (a + 1) * E]
                    for half in range(2):
                        sl = slice(j * PANEL + half * TB, j * PANEL + (half + 1) * TB)
                        pr = slice(half * 64, (half + 1) * 64)
                        nc.tensor.matmul(
                            ps1[pr, :], wh_t, xh_ch[(q, c)][:, sl],
                            start=(a == 0), stop=(a == KT - 1),
                        )
                        nc.tensor.matmul(
                            ps2[pr, :], wl_t, xh_ch[(q, c)][:, sl],
                            start=(a == 0), stop=False,
                        )
                        nc.tensor.matmul(
                            ps2[pr, :], wh_t, xl_ch[(q, c)][:, sl],
                            start=False, stop=(a == KT - 1),
                        )

                # ---- per-block epilogue ----
                for half in range(2):
                    pr = slice(half * 64, (half + 1) * 64)
                    t2 = work.tile([64, TB], f32, tag="t2")
                    nc.scalar.activation(
                        out=t2,
                        in_=ps2[pr, :],
                        func=mybir.ActivationFunctionType.Copy,
                        scale=LSCALE,
                    )
                    ltE = work.tile([64, TB], f32, tag="ltE")
                    nc.vector.tensor_add(ltE, t2, ps1[pr, :])

                    for tt in range(TB // 128):
                        t = (2 * q + half) * (TB // 128) + tt
                        ps_t = pspT.tile([128, E], f32, tag="ps_t")
                        nc.tensor.transpose(
                            ps_t, ltE[:, tt * 128 : (tt + 1) * 128], id_t
                        )

                        m8 = work.tile([128, 8], f32, tag="m8")
                        nc.vector.max(out=m8, in_=ps_t)
                        nc.vector.max_index(
                            stage_i[:, t * 8 : (t + 1) * 8], m8, ps_t
                        )

                        negm = work.tile([128, 1], f32, tag="negm")
                        nc.scalar.mul(negm, m8[:, 0:1], -1.0)
                        expw = work.tile([128, TOP_K], f32, tag="expw")
                        ssum = work.tile([128, 1], f32, tag="ssum")
                        nc.scalar.activation(
                            out=expw,
                            in_=m8[:, 0:TOP_K],
                            func=mybir.ActivationFunctionType.Exp,
                            bias=negm[:, 0:1],
                            scale=1.0,
                            accum_out=ssum[:, 0:1],
                        )
                        rsum = work.tile([128, 1], f32, tag="rsum")
                        nc.vector.reciprocal(rsum, ssum)
                        nc.vector.tensor_scalar_mul(
                            stage_w[:, t * TOP_K : (t + 1) * TOP_K],
                            expw,
                            rsum[:, 0:1],
                        )

                    # ---- per-half-panel output DMAs ----
                    nt_h = TB // 128  # 4 token tiles per half
                    c0 = (2 * q + half) * nt_h
                    nc.gpsimd.dma_start(
                        out=out_w[:, c0 * TOP_K : (c0 + nt_h) * TOP_K],
                        in_=stage_w[:, c0 * TOP_K : (c0 + nt_h) * TOP_K],
                    )
                    nc.gpsimd.dma_start(
                        out=out_i[:, c0 * 8 : (c0 + nt_h) * 8],
                        in_=stage_i[:, c0 * 8 : (c0 + nt_h) * 8].bitcast(i32),
                    )

    nc.compile()
    return nc


def _get_nc():
    if "nc" not in _CACHE:
        _CACHE["nc"] = _build()
    return _CACHE["nc"]


def _split_fp16(arr32):
    """arr32 (fp32) -> (hi fp16, lo fp16) with arr32 ~= hi + 2^-11 * lo."""
    hi = arr32.astype(np.float16)
    lo = ((arr32 - hi.astype(np.float32)) * 2048.0).astype(np.float16)
    return hi, lo


def kernel(hidden_states: np.ndarray, weight: np.ndarray, **_run_kwargs):
    x = np.ascontiguousarray(hidden_states, dtype=np.float32).reshape(T_FULL, H)
    w = np.ascontiguousarray(weight, dtype=np.float32)

    w_hi, w_lo = _split_fp16(w)  # [E, H] fp16
    # device layout [128, KT*E]: row p, col a*E+e  <-  W[e, a*128+p]
    def pack_w(wx):
        return np.ascontiguousarray(
            wx.T.reshape(KT, 128, E).transpose(1, 0, 2).reshape(128, KT * E)
        )

    whp = pack_w(w_hi)
    wlp = pack_w(w_lo)
    ident = np.eye(E, dtype=np.float32)

    def pack_x(xT16):
        # [H, T_CORE] -> [NP*NCH, 128, CH*PANEL]; chunk (q, c) block (p, j, t)
        # = xT16[c*CH*128 + j*128 + p, q*PANEL + t]
        v = xT16.reshape(NCH, CH, 128, NP, PANEL)
        return np.ascontiguousarray(
            v.transpose(3, 0, 2, 1, 4).reshape(NP * NCH, 128, CH * PANEL)
        )

    in_maps = []
    for c in range(N_CORES):
        shard = x[c * T_CORE : (c + 1) * T_CORE, :]  # [T_CORE, H]
        xT = np.ascontiguousarray(shard.T)  # [H, T_CORE] fp32
        xhs, xls = _split_fp16(xT)
        in_maps.append(
            {"xh": pack_x(xhs), "xl": pack_x(xls), "wh": whp, "wl": wlp, "ident": ident}
        )

    nc = _get_nc()
    res = run_bass_kernel_spmd(
        nc, in_maps, core_ids=list(range(N_CORES)), **_run_kwargs
    )

    idx_parts = []
    w_parts = []
    for c in range(N_CORES):
        r = res.results[c]
        si = r["out_i"].reshape(128, NTT, 8).transpose(1, 0, 2)[:, :, :TOP_K]
        sw = r["out_w"].reshape(128, NTT, TOP_K).transpose(1, 0, 2)
        idx_parts.append(si.reshape(T_CORE, TOP_K).astype(np.int32, copy=False))
        w_parts.append(sw.reshape(T_CORE, TOP_K))

    topk_idx = np.concatenate(idx_parts, axis=0)
    topk_weight = np.concatenate(w_parts, axis=0)
    if "trace" in _run_kwargs:
        return (topk_idx, topk_weight), res
    return topk_idx, topk_weight
